# revision 7
# baseline (speedup 1.0000x reference)
"""Expert-parallel MoE (top-2, E=8) for one Trainium2 chip (8 NeuronCores).

Contract: kernel(**inputs) takes the FULL unsharded inputs
  x  [4, 2048, 1024] f32,  Wr [1024, 8] f32,
  W1 [8, 1024, 2730] f32,  W2 [8, 2730, 1024] f32,  W3 [8, 1024, 2730] f32
and returns the FULL output [4, 2048, 1024] f32.

Sharding strategy (expert-parallel with balanced two-segment schedule):
  - The tiny router (softmax + top-2 over 8 experts) runs on host in fp32.
  - Each core runs an identical program of N1 + N2 token columns:
      segment 1 ("own"):     N1 columns of its primary expert, fp16.
      segment 2 ("foreign"): N2 columns of ONE overflow chunk from a
        surplus expert, computed with e3m4 fp8 operands (the ~3% of
        columns routed here tolerate fp8: measured rel-err 6.7e-3 total).
    (N1, N2) minimize N1+N2 s.t. every expert's token count splits into
    one N1-chunk plus <=8 total N2-chunks; for the reference routing this
    gives 1992 + 90 = 2082 columns/core vs 2135 with naive one-expert-
    per-core padding.
  - Per core the dense SwiGLU FFN runs out of SBUF-resident fp16 weights:
        Y^T = W2p^T @ (silu(W1p^T @ X^T) * (W3p^T @ X^T))
    fp16/fp8 operands, fp32 PSUM accumulation, fp32 output. The foreign
    segment's fp8 weights stream through small rotating SBUF buffers.
  - Host combine: out[tok] = sum_k gate[tok, k] * Y[core_k(tok)][pos_k].
"""

import copy
import json
import math
from contextlib import ExitStack

import numpy as np

# ---------------------------------------------------------------------------
# Walrus workaround: the neuronxcc walrus in this environment supports only
# ONE sync wait per instruction, while the Tile framework emits a final Drain
# carrying several.  Rewrite the serialized BIR: hoist extra waits into
# wait-only EventSemaphore instructions placed immediately before, on the
# same engine (the sequencer blocks on them in program order, so the
# semantics are unchanged).
# ---------------------------------------------------------------------------


def _split_multiwait_bir(bir_json):
    d = json.loads(bir_json)
    changed = False
    multi_update = []
    for fn in d.get("functions", []):

        def walk(block):
            nonlocal changed
            il = block.get("instructions")
            if il:
                new = []
                blk_changed = False
                for i in il:
                    si = i.get("sync_info") or {}
                    ws = si.get("on_wait") or []
                    if len(ws) > 1:
                        for j, w in enumerate(ws[:-1]):
                            new.append(
                                {
                                    "debug": i.get("debug"),
                                    "engine": i["engine"],
                                    "ins": [],
                                    "outs": [],
                                    "name": f"{i['name']}_xw{j}",
                                    "opcode": "EventSemaphore",
                                    "sync_info": {"on_update": [], "on_wait": [w]},
                                }
                            )
                        i = copy.deepcopy(i)
                        i["sync_info"]["on_wait"] = [ws[-1]]
                        blk_changed = True
                    us = (i.get("sync_info") or {}).get("on_update") or []
                    if len(us) > 1:
                        multi_update.append((i.get("name"), i.get("opcode")))
                    new.append(i)
                if blk_changed:
                    block["instructions"] = new
                    changed = True
            for b in block.get("blocks", []) or []:
                walk(b)

        walk(fn)

        # Trim the post-drain barrier/sem-clear tail of the TileContext end
        # block (~5-10 us of EVSEM butterfly).  The Drain already guarantees
        # all output DMAs completed; sems are re-initialized by the preamble
        # on the next execution (verified by back-to-back runs).
        def trim(block):
            nonlocal changed
            il = block.get("instructions")
            if il and block.get("name", "").endswith("_end"):
                last_drain = None
                for idx, i in enumerate(il):
                    if i.get("opcode") == "Drain" and i.get("engine") == "SP":
                        last_drain = idx
                        break
                if last_drain is not None and last_drain + 1 < len(il):
                    block["instructions"] = il[: last_drain + 1]
                    changed = True
            for b in block.get("blocks", []) or []:
                trim(b)

        trim(fn)
    if multi_update:
        raise RuntimeError(f"multi-update instructions unsupported: {multi_update[:5]}")
    if not changed:
        return bir_json
    return json.dumps(d).encode()


_patched = False


def _install_bir_patch():
    global _patched
    if _patched:
        return
    import concourse.bass2jax as b2j

    orig = b2j.compile_bir_kernel

    def patched(bir_json, tmpdir, neff_name="file.neff"):
        return orig(_split_multiwait_bir(bir_json), tmpdir, neff_name)

    b2j.compile_bir_kernel = patched
    _patched = True


_install_bir_patch()

import concourse.bass as bass
import concourse.mybir as mybir
import concourse.tile as tile
from concourse.bass_utils import run_bass_kernel_spmd

D = 1024
E = 8
TOP_K = 2
H = 2730
HP = 2816  # H padded to 22 * 128
DT = mybir.dt.float16
NP_DT = np.float16
DT8 = mybir.dt.float8e3
NP_DT8 = mybir.dt.np(mybir.dt.float8e3)  # ml_dtypes.float8_e3m4
D_TILES = D // 128  # 8
H_TILES = HP // 128  # 22

# fp8 (e3m4) scaling for the foreign segment.  Weights/activations are
# scaled into e3m4's sweet spot; the silu/copy activations descale by
# 1/(SX*SW) = 2^-9 (exact), and the host descales the output by 1/SW2.
SX = 2.0
SW = 256.0
SW2 = 512.0
DESCALE = 1.0 / (SX * SW)  # 2^-9, applied on-device pre-silu
WARMUP_MMS = 28  # ~3us of N=128 matmuls: covers HAM clock ramp + input DMA


def _plan_blocks(C):
    blocks = []
    rem = C
    while rem >= 512:
        blocks.append(512)
        rem -= 512
    if rem:
        blocks.append(rem)
    return blocks


def _solve_split(counts):
    """Pick (N1, N2): per core N1 own columns + one foreign chunk of <= N2
    columns, minimizing N1+N2 s.t. all surpluses pack into <= E chunks."""
    best = None
    lo, hi = int(min(counts)), int(max(counts))
    for N1 in range(lo, hi + 1):
        surp = [int(c) - N1 for c in counts if int(c) > N1]
        if not surp:
            cand = (N1, 0)
        else:
            a, b = 1, max(surp)
            while a < b:
                mid = (a + b) // 2
                if sum(-(-s // mid) for s in surp) <= E:
                    b = mid
                else:
                    a = mid + 1
            cand = (N1, a)
        T = cand[0] + cand[1]
        if best is None or T < best[0]:
            best = (T, cand)
    return best[1]


def _build_nc(N1, N2):
    blocks = _plan_blocks(N1)
    nc = bass.Bass()
    f32 = mybir.dt.float32

    xt = nc.dram_tensor("xt", [D, N1], DT, kind="ExternalInput")
    w1 = nc.dram_tensor("w1", [D, HP], DT, kind="ExternalInput")
    w3 = nc.dram_tensor("w3", [D, HP], DT, kind="ExternalInput")
    w2 = nc.dram_tensor("w2", [HP, D], DT, kind="ExternalInput")
    if N2:
        xf = nc.dram_tensor("xf", [D, N2], DT8, kind="ExternalInput")
        w1f = nc.dram_tensor("w1f", [D, HP], DT8, kind="ExternalInput")
        w3f = nc.dram_tensor("w3f", [D, HP], DT8, kind="ExternalInput")
        w2f = nc.dram_tensor("w2f", [HP, D], DT8, kind="ExternalInput")
    yt = nc.dram_tensor("yt", [D, N1 + N2], f32, kind="ExternalOutput")

    with tile.TileContext(nc) as tc, ExitStack() as ctx:
        wpool = ctx.enter_context(tc.tile_pool(name="w", bufs=1))
        xpool = ctx.enter_context(tc.tile_pool(name="x", bufs=2))
        hpool = ctx.enter_context(tc.tile_pool(name="h", bufs=1))
        spool = ctx.enter_context(tc.tile_pool(name="s", bufs=1))
        ypool = ctx.enter_context(tc.tile_pool(name="y", bufs=1))
        psA = ctx.enter_context(tc.tile_pool(name="psA", bufs=4, space="PSUM"))
        psY = ctx.enter_context(tc.tile_pool(name="psY", bufs=2, space="PSUM"))
        if N2:
            # rotating stream buffers for the foreign fp8 weights
            wfpool = ctx.enter_context(tc.tile_pool(name="wf", bufs=4))
            w2fpool = ctx.enter_context(tc.tile_pool(name="w2f", bufs=2))
            xfpool = ctx.enter_context(tc.tile_pool(name="xf", bufs=1))

        # DRAM views with the 128-partition dim split out so one dma_start
        # covers all row-tiles of a column chunk (each dma_start costs
        # ~650 ns of serial sequencer dispatch: fewer + bigger wins).
        xt_v = xt.rearrange("(d p) c -> p d c", p=128)
        w1_v = w1.rearrange("(d p) h -> p d h", p=128)
        w3_v = w3.rearrange("(d p) h -> p d h", p=128)
        w2_v = w2.rearrange("(h p) d -> p h d", p=128)
        if N2:
            xf_v = xf.rearrange("(d p) c -> p d c", p=128)
            w1f_v = w1f.rearrange("(d p) h -> p d h", p=128)
            w3f_v = w3f.rearrange("(d p) h -> p d h", p=128)
            w2f_v = w2f.rearrange("(h p) d -> p h d", p=128)

        # Dependency-free warmup matmuls: keep the PE busy from t=0 so the
        # HAM clock gate opens (1.2 -> 2.4 GHz) and the input DMAs have a
        # busy window to land in before the first real matmul group.
        warm = ypool.tile([128, 128], DT, tag="warm")
        wps = psA.tile([128, 512], f32, tag="psA")
        for _ in range(WARMUP_MMS):
            nc.tensor.matmul(
                wps[:, :128], lhsT=warm[:, :128], rhs=warm[:, :128], start=True, stop=True
            )
        # warm is read uninitialized on purpose: the products land in a PSUM
        # tile that is never consumed, and skipping the memset removes the
        # DVE-preamble dependency so the PE warms from the start.
        nc.vector.memset(warm[:], 0.0)

        def load_x(off, TB, eng):
            x_sb = xpool.tile([128, D_TILES, TB], DT, tag="x")
            eng.dma_start(x_sb[:], xt_v[:, :, off : off + TB])
            return x_sb

        # First token block loads before the weight stream (split across the
        # two HWDGE rings) so the first matmul group unblocks early.
        x_pre = xpool.tile([128, D_TILES, blocks[0]], DT, tag="x")
        nc.gpsimd.dma_start(
            x_pre[:, : D_TILES // 2], xt_v[:, : D_TILES // 2, 0 : blocks[0]]
        )
        nc.scalar.dma_start(
            x_pre[:, D_TILES // 2 :], xt_v[:, D_TILES // 2 :, 0 : blocks[0]]
        )

        # SBUF-resident own weights, streamed in phase-A consumption order.
        w1_sb = wpool.tile([128, D_TILES, HP], DT, tag="w1")
        w3_sb = wpool.tile([128, D_TILES, HP], DT, tag="w3")
        w2_sb = wpool.tile([128, H_TILES, D], DT, tag="w2")
        hc_off = 0
        for hc in [128, 128, 256, 256] + [512] * 4:
            sl = slice(hc_off, hc_off + hc)
            nc.sync.dma_start(w1_sb[:, :, sl], w1_v[:, :, sl])
            nc.sync.dma_start(w3_sb[:, :, sl], w3_v[:, :, sl])
            hc_off += hc
        assert hc_off == HP
        for h_i in range(0, H_TILES, 6):
            nh = min(6, H_TILES - h_i)
            nc.sync.dma_start(w2_sb[:, h_i : h_i + nh], w2_v[:, h_i : h_i + nh, :])

        # Foreign fp8 inputs: x upfront (tiny); weights as a rotating
        # stream on the sync ring, queued behind the own weights.  The
        # first `bufs` chunks prefetch during the own segment's compute;
        # later triggers self-throttle on buffer reuse.
        if N2:
            xf_sb = xfpool.tile([128, D_TILES, N2], DT8, tag="xf")
            nc.gpsimd.dma_start(xf_sb[:], xf_v[:])
            wf_tiles = []
            for p in range(0, H_TILES, 2):
                nh = min(2, H_TILES - p)
                sl = slice(p * 128, (p + nh) * 128)
                t1 = wfpool.tile([128, D_TILES, nh * 128], DT8, tag="wf")
                nc.sync.dma_start(t1, w1f_v[:, :, sl])
                t3 = wfpool.tile([128, D_TILES, nh * 128], DT8, tag="wf")
                nc.sync.dma_start(t3, w3f_v[:, :, sl])
                wf_tiles.append((t1, t3))
            w2f_tiles = []
            for p in range(0, D_TILES, 2):
                sl = slice(p * 128, (p + 2) * 128)
                t2 = w2fpool.tile([128, H_TILES, 256], DT8, tag="w2f")
                nc.sync.dma_start(t2, w2f_v[:, :, sl])
                w2f_tiles.append(t2)

        def phase_a(x_sb, TB, h_sb, h_i, w1l, w3l, act_scale):
            ps1 = psA.tile([128, TB], f32, tag="psA")
            for d_i in range(D_TILES):
                nc.tensor.matmul(
                    ps1,
                    lhsT=w1l(d_i),
                    rhs=x_sb[:, d_i],
                    start=(d_i == 0),
                    stop=(d_i == D_TILES - 1),
                )
            ps3 = psA.tile([128, TB], f32, tag="psA")
            for d_i in range(D_TILES):
                nc.tensor.matmul(
                    ps3,
                    lhsT=w3l(d_i),
                    rhs=x_sb[:, d_i],
                    start=(d_i == 0),
                    stop=(d_i == D_TILES - 1),
                )
            sil = spool.tile([128, TB], f32, tag="sil")
            nc.scalar.activation(sil, ps1, mybir.ActivationFunctionType.Silu, scale=act_scale)
            if act_scale != 1.0:
                sil3 = spool.tile([128, TB], f32, tag="sil3")
                nc.scalar.activation(
                    sil3, ps3, mybir.ActivationFunctionType.Copy, scale=act_scale
                )
                nc.vector.tensor_mul(h_sb[:, h_i], sil, sil3)
            else:
                nc.vector.tensor_mul(h_sb[:, h_i], sil, ps3)

        off = 0
        for bi, TB in enumerate(blocks):
            x_sb = x_pre if bi == 0 else load_x(off, TB, nc.gpsimd)

            # Phase A: H^T[:, block] = silu(W1^T X^T) * (W3^T X^T), fp16.
            h_sb = hpool.tile([128, H_TILES, TB], DT, tag="h")
            for h_i in range(H_TILES):
                phase_a(
                    x_sb,
                    TB,
                    h_sb,
                    h_i,
                    lambda d_i, h_i=h_i: w1_sb[:, d_i, h_i * 128 : (h_i + 1) * 128],
                    lambda d_i, h_i=h_i: w3_sb[:, d_i, h_i * 128 : (h_i + 1) * 128],
                    1.0,
                )

            # Phase B: Y^T[:, block] = W2^T @ H^T.
            for m_i in range(D_TILES):
                psy = psY.tile([128, TB], f32, tag="psY")
                for h_i in range(H_TILES):
                    nc.tensor.matmul(
                        psy,
                        lhsT=w2_sb[:, h_i, m_i * 128 : (m_i + 1) * 128],
                        rhs=h_sb[:, h_i],
                        start=(h_i == 0),
                        stop=(h_i == H_TILES - 1),
                    )
                y_sb = ypool.tile([128, TB], f32, tag="y")
                nc.vector.tensor_copy(y_sb, psy)
                nc.scalar.dma_start(yt[m_i * 128 : (m_i + 1) * 128, off : off + TB], y_sb)
            off += TB

        # Foreign segment: N2 columns of the overflow expert in e3m4.
        if N2:
            hf = hpool.tile([128, H_TILES, N2], DT8, tag="hf")
            for h_i in range(H_TILES):
                p, sub = divmod(h_i, 2)
                t1, t3 = wf_tiles[p]
                phase_a(
                    xf_sb,
                    N2,
                    hf,
                    h_i,
                    lambda d_i, t1=t1, sub=sub: t1[:, d_i, sub * 128 : (sub + 1) * 128],
                    lambda d_i, t3=t3, sub=sub: t3[:, d_i, sub * 128 : (sub + 1) * 128],
                    DESCALE,
                )
            for m_i in range(D_TILES):
                p, sub = divmod(m_i, 2)
                psy = psY.tile([128, N2], f32, tag="psY")
                for h_i in range(H_TILES):
                    nc.tensor.matmul(
                        psy,
                        lhsT=w2f_tiles[p][:, h_i, sub * 128 : (sub + 1) * 128],
                        rhs=hf[:, h_i],
                        start=(h_i == 0),
                        stop=(h_i == H_TILES - 1),
                    )
                y_sb = ypool.tile([128, N2], f32, tag="y")
                nc.vector.tensor_copy(y_sb, psy)
                nc.scalar.dma_start(yt[m_i * 128 : (m_i + 1) * 128, N1 : N1 + N2], y_sb)

    return nc


def _route(flat, Wr):
    logits = flat @ Wr
    m = logits.max(-1, keepdims=True)
    p = np.exp(logits - m)
    p /= p.sum(-1, keepdims=True)
    topi = np.argsort(-p, axis=-1)[:, :TOP_K]
    topv = np.take_along_axis(p, topi, -1)
    toks, gates = [], []
    for e in range(E):
        sel = topi == e
        rows = np.where(sel.any(-1))[0]
        toks.append(rows)
        gates.append((topv * sel)[rows].sum(-1))
    return toks, gates


_NC_CACHE = {}


def kernel(x, Wr, W1, W2, W3, _trace=False, _result=None):
    x = np.asarray(x)
    Wr = np.asarray(Wr, dtype=np.float32)
    W1 = np.asarray(W1)
    W2 = np.asarray(W2)
    W3 = np.asarray(W3)
    Bx, Tx, Dx = x.shape
    N = Bx * Tx
    flat = np.ascontiguousarray(x.reshape(N, Dx).astype(np.float32))

    toks, gates = _route(flat, Wr)
    counts = np.array([len(t) for t in toks])
    N1, N2 = _solve_split(counts)
    N1 = max(N1, 128)

    # Foreign chunks: per expert, tokens beyond N1 split into <=N2 pieces.
    chunks = []
    for e in range(E):
        rest = toks[e][N1:]
        for s in range(0, len(rest), max(N2, 1)):
            chunks.append((e, rest[s : s + N2]))
    assert len(chunks) <= E, (counts, N1, N2)

    flat16 = flat.astype(NP_DT)
    flat8 = (flat * SX).astype(NP_DT8)
    in_maps = []
    for c in range(E):
        own = toks[c][:N1]
        xte = np.zeros((D, N1), NP_DT)
        xte[:, : len(own)] = flat16[own].T
        w1e = np.zeros((D, HP), NP_DT)
        w1e[:, :H] = W1[c].astype(NP_DT)
        w3e = np.zeros((D, HP), NP_DT)
        w3e[:, :H] = W3[c].astype(NP_DT)
        w2e = np.zeros((HP, D), NP_DT)
        w2e[:H, :] = W2[c].astype(NP_DT)
        im = {"xt": xte, "w1": w1e, "w3": w3e, "w2": w2e}
        if N2:
            xfe = np.zeros((D, N2), NP_DT8)
            w1fe = np.zeros((D, HP), NP_DT8)
            w3fe = np.zeros((D, HP), NP_DT8)
            w2fe = np.zeros((HP, D), NP_DT8)
            if c < len(chunks):
                fe, ftoks = chunks[c]
                xfe[:, : len(ftoks)] = flat8[ftoks].T
                w1fe[:, :H] = (W1[fe] * SW).astype(NP_DT8)
                w3fe[:, :H] = (W3[fe] * SW).astype(NP_DT8)
                w2fe[:H, :] = (W2[fe] * SW2).astype(NP_DT8)
            im.update({"xf": xfe, "w1f": w1fe, "w3f": w3fe, "w2f": w2fe})
        in_maps.append(im)

    key = (N1, N2)
    if key not in _NC_CACHE:
        _NC_CACHE[key] = _build_nc(N1, N2)
    nc = _NC_CACHE[key]

    res = run_bass_kernel_spmd(nc, in_maps, list(range(E)), trace=_trace)
    if _result is not None:
        _result.append(res)

    Y = np.stack([res.results[c]["yt"] for c in range(E)])  # [E, D, N1+N2]
    out = np.zeros((N, D), np.float32)
    for e in range(E):
        own = toks[e][:N1]
        out[own] += gates[e][: len(own), None] * Y[e, :, : len(own)].T
    if N2:
        # gates for foreign tokens: walk each expert's overflow in chunk order
        pos = {e: N1 for e in range(E)}
        for c, (fe, ftoks) in enumerate(chunks):
            k = len(ftoks)
            g = gates[fe][pos[fe] : pos[fe] + k]
            pos[fe] += k
            out[ftoks] += (g[:, None] * Y[c, :, N1 : N1 + k].T) / SW2
    return out.reshape(Bx, Tx, Dx).astype(x.dtype)


# revision 16
# speedup vs baseline: 1.0009x; 1.0009x over previous
"""Expert-parallel MoE (top-2, E=8) for one Trainium2 chip (8 NeuronCores).

Contract: kernel(**inputs) takes the FULL unsharded inputs
  x  [4, 2048, 1024] f32,  Wr [1024, 8] f32,
  W1 [8, 1024, 2730] f32,  W2 [8, 2730, 1024] f32,  W3 [8, 1024, 2730] f32
and returns the FULL output [4, 2048, 1024] f32.

Sharding strategy (expert-parallel with balanced two-segment schedule):
  - The tiny router (softmax + top-2 over 8 experts) runs on host in fp32.
  - Each core runs an identical program of N1 + N2 token columns:
      segment 1 ("own"):     N1 columns of its primary expert, fp16.
      segment 2 ("foreign"): N2 columns of ONE overflow chunk from a
        surplus expert, computed with e3m4 fp8 operands (the ~3% of
        columns routed here tolerate fp8: measured rel-err 6.7e-3 total).
    (N1, N2) minimize N1+N2 s.t. every expert's token count splits into
    one N1-chunk plus <=8 total N2-chunks; for the reference routing this
    gives 1992 + 90 = 2082 columns/core vs 2135 with naive one-expert-
    per-core padding.
  - Per core the dense SwiGLU FFN runs out of SBUF-resident fp16 weights:
        Y^T = W2p^T @ (silu(W1p^T @ X^T) * (W3p^T @ X^T))
    fp16/fp8 operands, fp32 PSUM accumulation, fp32 output. The foreign
    segment's fp8 weights stream through small rotating SBUF buffers.
  - Host combine: out[tok] = sum_k gate[tok, k] * Y[core_k(tok)][pos_k].
"""

import copy
import json
import math
from contextlib import ExitStack

import numpy as np

# ---------------------------------------------------------------------------
# Walrus workaround: the neuronxcc walrus in this environment supports only
# ONE sync wait per instruction, while the Tile framework emits a final Drain
# carrying several.  Rewrite the serialized BIR: hoist extra waits into
# wait-only EventSemaphore instructions placed immediately before, on the
# same engine (the sequencer blocks on them in program order, so the
# semantics are unchanged).
# ---------------------------------------------------------------------------


def _split_multiwait_bir(bir_json):
    d = json.loads(bir_json)
    changed = False
    multi_update = []
    for fn in d.get("functions", []):

        def walk(block):
            nonlocal changed
            il = block.get("instructions")
            if il:
                new = []
                blk_changed = False
                for i in il:
                    si = i.get("sync_info") or {}
                    ws = si.get("on_wait") or []
                    if len(ws) > 1:
                        for j, w in enumerate(ws[:-1]):
                            new.append(
                                {
                                    "debug": i.get("debug"),
                                    "engine": i["engine"],
                                    "ins": [],
                                    "outs": [],
                                    "name": f"{i['name']}_xw{j}",
                                    "opcode": "EventSemaphore",
                                    "sync_info": {"on_update": [], "on_wait": [w]},
                                }
                            )
                        i = copy.deepcopy(i)
                        i["sync_info"]["on_wait"] = [ws[-1]]
                        blk_changed = True
                    us = (i.get("sync_info") or {}).get("on_update") or []
                    if len(us) > 1:
                        multi_update.append((i.get("name"), i.get("opcode")))
                    new.append(i)
                if blk_changed:
                    block["instructions"] = new
                    changed = True
            for b in block.get("blocks", []) or []:
                walk(b)

        walk(fn)

        # Trim the post-drain barrier/sem-clear tail of the TileContext end
        # block (~5-10 us of EVSEM butterfly).  The Drain already guarantees
        # all output DMAs completed; sems are re-initialized by the preamble
        # on the next execution (verified by back-to-back runs).
        def trim(block):
            nonlocal changed
            il = block.get("instructions")
            if il and block.get("name", "").endswith("_end"):
                last_drain = None
                for idx, i in enumerate(il):
                    if i.get("opcode") == "Drain" and i.get("engine") == "SP":
                        last_drain = idx
                        break
                if last_drain is not None and last_drain + 1 < len(il):
                    block["instructions"] = il[: last_drain + 1]
                    changed = True
            for b in block.get("blocks", []) or []:
                trim(b)

        trim(fn)
    if multi_update:
        raise RuntimeError(f"multi-update instructions unsupported: {multi_update[:5]}")
    if not changed:
        return bir_json
    return json.dumps(d).encode()


_patched = False


def _install_bir_patch():
    global _patched
    if _patched:
        return
    import concourse.bass2jax as b2j

    orig = b2j.compile_bir_kernel

    def patched(bir_json, tmpdir, neff_name="file.neff"):
        return orig(_split_multiwait_bir(bir_json), tmpdir, neff_name)

    b2j.compile_bir_kernel = patched
    _patched = True


_install_bir_patch()

import concourse.bass as bass
import concourse.mybir as mybir
import concourse.tile as tile
from concourse.bass_utils import run_bass_kernel_spmd

D = 1024
E = 8
TOP_K = 2
H = 2730
HP = 2816  # H padded to 22 * 128
DT = mybir.dt.float16
NP_DT = np.float16
DT8 = mybir.dt.float8e3
NP_DT8 = mybir.dt.np(mybir.dt.float8e3)  # ml_dtypes.float8_e3m4
D_TILES = D // 128  # 8
H_TILES = HP // 128  # 22

# fp8 (e3m4) scaling for the foreign segment.  Weights/activations are
# scaled into e3m4's sweet spot; the silu/copy activations descale by
# 1/(SX*SW) = 2^-9 (exact), and the host descales the output by 1/SW2.
SX = 2.0
SW = 256.0
SW2 = 512.0
DESCALE = 1.0 / (SX * SW)  # 2^-9, applied on-device pre-silu
# First input DMAs land ~13us in (9us preamble+trigger floor + ~4us for the
# first x/w chunks at the measured ~300 GB/s aggregate queue rate); warmup
# matmuls bridge exactly that window so the PE is busy and HAM-warm.
WARMUP_MMS = 36  # N=256 each: ~16 cold (213ns) + 20 warm (107ns) ~= 5.5us


def _plan_blocks(C):
    blocks = []
    rem = C
    while rem >= 512:
        blocks.append(512)
        rem -= 512
    if rem:
        blocks.append(rem)
    return blocks


def _solve_split(counts):
    """Pick (N1, N2): per core N1 own columns + one foreign chunk of <= N2
    columns, minimizing N1+N2 s.t. all surpluses pack into <= E chunks."""
    best = None
    lo, hi = int(min(counts)), int(max(counts))
    for N1 in range(lo, hi + 1):
        surp = [int(c) - N1 for c in counts if int(c) > N1]
        if not surp:
            cand = (N1, 0)
        else:
            a, b = 1, max(surp)
            while a < b:
                mid = (a + b) // 2
                if sum(-(-s // mid) for s in surp) <= E:
                    b = mid
                else:
                    a = mid + 1
            cand = (N1, a)
        T = cand[0] + cand[1]
        if best is None or T < best[0]:
            best = (T, cand)
    return best[1]


def _build_nc(N1, N2):
    blocks = _plan_blocks(N1)
    nc = bass.Bass()
    f32 = mybir.dt.float32

    xt = nc.dram_tensor("xt", [D, N1], DT, kind="ExternalInput")
    w1 = nc.dram_tensor("w1", [D, HP], DT, kind="ExternalInput")
    w3 = nc.dram_tensor("w3", [D, HP], DT, kind="ExternalInput")
    w2 = nc.dram_tensor("w2", [HP, D], DT, kind="ExternalInput")
    if N2:
        xf = nc.dram_tensor("xf", [D, N2], DT8, kind="ExternalInput")
        w1f = nc.dram_tensor("w1f", [D, HP], DT8, kind="ExternalInput")
        w3f = nc.dram_tensor("w3f", [D, HP], DT8, kind="ExternalInput")
        w2f = nc.dram_tensor("w2f", [HP, D], DT8, kind="ExternalInput")
    yt = nc.dram_tensor("yt", [D, N1 + N2], f32, kind="ExternalOutput")

    with tile.TileContext(nc) as tc, ExitStack() as ctx:
        wpool = ctx.enter_context(tc.tile_pool(name="w", bufs=1))
        xpool = ctx.enter_context(tc.tile_pool(name="x", bufs=2))
        hpool = ctx.enter_context(tc.tile_pool(name="h", bufs=1))
        spool = ctx.enter_context(tc.tile_pool(name="s", bufs=1))
        ypool = ctx.enter_context(tc.tile_pool(name="y", bufs=1))
        psA = ctx.enter_context(tc.tile_pool(name="psA", bufs=4, space="PSUM"))
        psY = ctx.enter_context(tc.tile_pool(name="psY", bufs=2, space="PSUM"))
        if N2:
            # rotating stream buffers for the foreign fp8 weights
            wfpool = ctx.enter_context(tc.tile_pool(name="wf", bufs=5))
            w2fpool = ctx.enter_context(tc.tile_pool(name="w2f", bufs=2))
            xfpool = ctx.enter_context(tc.tile_pool(name="xf", bufs=1))

        # DRAM views with the 128-partition dim split out so one dma_start
        # covers all row-tiles of a column chunk (each dma_start costs
        # ~650 ns of serial sequencer dispatch: fewer + bigger wins).
        xt_v = xt.rearrange("(d p) c -> p d c", p=128)
        w1_v = w1.rearrange("(d p) h -> p d h", p=128)
        w3_v = w3.rearrange("(d p) h -> p d h", p=128)
        w2_v = w2.rearrange("(h p) d -> p h d", p=128)
        if N2:
            xf_v = xf.rearrange("(d p) c -> p d c", p=128)
            w1f_v = w1f.rearrange("(d p) h -> p d h", p=128)
            w3f_v = w3f.rearrange("(d p) h -> p d h", p=128)
            w2f_v = w2f.rearrange("(h p) d -> p h d", p=128)

        # Dependency-free warmup matmuls: keep the PE busy from t=0 so the
        # HAM clock gate opens (1.2 -> 2.4 GHz) and the input DMAs have a
        # busy window to land in before the first real matmul group.
        warm = ypool.tile([128, 256], DT, tag="warm")
        wps = psA.tile([128, 512], f32, tag="psA")
        for _ in range(WARMUP_MMS):
            nc.tensor.matmul(
                wps[:, :256], lhsT=warm[:, :128], rhs=warm[:, :256], start=True, stop=True
            )
        # warm is read uninitialized on purpose: the products land in a PSUM
        # tile that is never consumed, and skipping the memset removes the
        # DVE-preamble dependency so the PE warms from the start.
        nc.vector.memset(warm[:], 0.0)

        def load_x(off, TB, eng):
            x_sb = xpool.tile([128, D_TILES, TB], DT, tag="x")
            eng.dma_start(x_sb[:], xt_v[:, :, off : off + TB])
            return x_sb

        # Foreign x first on the gpsimd ring: tiny, and it must not queue
        # behind the self-throttled per-block x loads (the first foreign
        # phase-A slice runs right after own block 0).
        if N2:
            xf_sb = xfpool.tile([128, D_TILES, N2], DT8, tag="xf")
            nc.gpsimd.dma_start(xf_sb[:], xf_v[:])

        # First token block loads before the weight stream (split across the
        # two HWDGE rings) so the first matmul group unblocks early.
        x_pre = xpool.tile([128, D_TILES, blocks[0]], DT, tag="x")
        nc.gpsimd.dma_start(
            x_pre[:, : D_TILES // 2], xt_v[:, : D_TILES // 2, 0 : blocks[0]]
        )
        nc.scalar.dma_start(
            x_pre[:, D_TILES // 2 :], xt_v[:, D_TILES // 2 :, 0 : blocks[0]]
        )

        # SBUF-resident own weights, streamed in phase-A consumption order.
        w1_sb = wpool.tile([128, D_TILES, HP], DT, tag="w1")
        w3_sb = wpool.tile([128, D_TILES, HP], DT, tag="w3")
        w2_sb = wpool.tile([128, H_TILES, D], DT, tag="w2")
        hc_off = 0
        for hc in [128, 128, 256, 256] + [512] * 4:
            sl = slice(hc_off, hc_off + hc)
            nc.sync.dma_start(w1_sb[:, :, sl], w1_v[:, :, sl])
            nc.sync.dma_start(w3_sb[:, :, sl], w3_v[:, :, sl])
            hc_off += hc
        assert hc_off == HP
        for h_i in range(0, H_TILES, 6):
            nh = min(6, H_TILES - h_i)
            nc.sync.dma_start(w2_sb[:, h_i : h_i + nh], w2_v[:, h_i : h_i + nh, :])

        # Foreign fp8 weight stream on the sync ring, queued behind the own
        # weights.  Chunks cover 2 h-tiles each; the foreign phase-A work is
        # interleaved between own blocks below, so the rotating buffers are
        # consumed (and the queue advances) throughout the own segment
        # instead of bursting at the end.
        if N2:
            wf_tiles = []
            for p in range(0, H_TILES, 2):
                nh = min(2, H_TILES - p)
                sl = slice(p * 128, (p + nh) * 128)
                t1 = wfpool.tile([128, D_TILES, nh * 128], DT8, tag="wf")
                nc.sync.dma_start(t1, w1f_v[:, :, sl])
                t3 = wfpool.tile([128, D_TILES, nh * 128], DT8, tag="wf")
                nc.sync.dma_start(t3, w3f_v[:, :, sl])
                wf_tiles.append((t1, t3))
            # w2f rides the scalar ring for the two prefetched chunks; the
            # last two go on the gpsimd ring AFTER the own loop — their
            # buffer-reuse waits block the queue head, so they must sit on
            # a ring with no later traffic (scalar carries all y-output
            # DMAs: a blocked w2f trigger there would deadlock phase B).
            w2f_tiles = []

            def w2f_load(p, eng):
                sl = slice(p * 256, (p + 1) * 256)
                t2 = w2fpool.tile([128, H_TILES, 256], DT8, tag="w2f")
                eng.dma_start(t2, w2f_v[:, :, sl])
                w2f_tiles.append(t2)

            w2f_load(0, nc.scalar)
            w2f_load(1, nc.scalar)

        def phase_a(x_sb, TB, h_sb, h_i, w1l, w3l, act_scale):
            ps1 = psA.tile([128, TB], f32, tag="psA")
            for d_i in range(D_TILES):
                nc.tensor.matmul(
                    ps1,
                    lhsT=w1l(d_i),
                    rhs=x_sb[:, d_i],
                    start=(d_i == 0),
                    stop=(d_i == D_TILES - 1),
                )
            ps3 = psA.tile([128, TB], f32, tag="psA")
            for d_i in range(D_TILES):
                nc.tensor.matmul(
                    ps3,
                    lhsT=w3l(d_i),
                    rhs=x_sb[:, d_i],
                    start=(d_i == 0),
                    stop=(d_i == D_TILES - 1),
                )
            sil = spool.tile([128, TB], f32, tag="sil")
            nc.scalar.activation(sil, ps1, mybir.ActivationFunctionType.Silu, scale=act_scale)
            if act_scale != 1.0:
                sil3 = spool.tile([128, TB], f32, tag="sil3")
                nc.scalar.activation(
                    sil3, ps3, mybir.ActivationFunctionType.Copy, scale=act_scale
                )
                nc.vector.tensor_mul(h_sb[:, h_i], sil, sil3)
            else:
                nc.vector.tensor_mul(h_sb[:, h_i], sil, ps3)

        # Foreign phase-A h-tiles, sliced evenly across the own blocks.
        if N2:
            hf = hpool.tile([128, H_TILES, N2], DT8, tag="hf")
            nb = len(blocks)
            base, rem = divmod(H_TILES, nb)
            fslices = []
            start = 0
            for bi in range(nb):
                n = base + (1 if bi < rem else 0)
                fslices.append(range(start, start + n))
                start += n

            def foreign_a(h_range):
                for h_i in h_range:
                    p, sub = divmod(h_i, 2)
                    t1, t3 = wf_tiles[p]
                    phase_a(
                        xf_sb,
                        N2,
                        hf,
                        h_i,
                        lambda d_i, t1=t1, sub=sub: t1[
                            :, d_i, sub * 128 : (sub + 1) * 128
                        ],
                        lambda d_i, t3=t3, sub=sub: t3[
                            :, d_i, sub * 128 : (sub + 1) * 128
                        ],
                        DESCALE,
                    )

        off = 0
        for bi, TB in enumerate(blocks):
            x_sb = x_pre if bi == 0 else load_x(off, TB, nc.gpsimd)

            # Phase A: H^T[:, block] = silu(W1^T X^T) * (W3^T X^T), fp16.
            h_sb = hpool.tile([128, H_TILES, TB], DT, tag="h")
            for h_i in range(H_TILES):
                phase_a(
                    x_sb,
                    TB,
                    h_sb,
                    h_i,
                    lambda d_i, h_i=h_i: w1_sb[:, d_i, h_i * 128 : (h_i + 1) * 128],
                    lambda d_i, h_i=h_i: w3_sb[:, d_i, h_i * 128 : (h_i + 1) * 128],
                    1.0,
                )

            # Phase B: Y^T[:, block] = W2^T @ H^T.
            for m_i in range(D_TILES):
                psy = psY.tile([128, TB], f32, tag="psY")
                for h_i in range(H_TILES):
                    nc.tensor.matmul(
                        psy,
                        lhsT=w2_sb[:, h_i, m_i * 128 : (m_i + 1) * 128],
                        rhs=h_sb[:, h_i],
                        start=(h_i == 0),
                        stop=(h_i == H_TILES - 1),
                    )
                y_sb = ypool.tile([128, TB], f32, tag="y")
                nc.vector.tensor_copy(y_sb, psy)
                nc.scalar.dma_start(yt[m_i * 128 : (m_i + 1) * 128, off : off + TB], y_sb)
            off += TB
            if N2:
                foreign_a(fslices[bi])

        # Foreign phase B: N2 columns of the overflow expert in e3m4.
        if N2:
            w2f_load(2, nc.gpsimd)
            w2f_load(3, nc.gpsimd)
            for m_i in range(D_TILES):
                p, sub = divmod(m_i, 2)
                psy = psY.tile([128, N2], f32, tag="psY")
                for h_i in range(H_TILES):
                    nc.tensor.matmul(
                        psy,
                        lhsT=w2f_tiles[p][:, h_i, sub * 128 : (sub + 1) * 128],
                        rhs=hf[:, h_i],
                        start=(h_i == 0),
                        stop=(h_i == H_TILES - 1),
                    )
                y_sb = ypool.tile([128, N2], f32, tag="y")
                nc.vector.tensor_copy(y_sb, psy)
                nc.scalar.dma_start(yt[m_i * 128 : (m_i + 1) * 128, N1 : N1 + N2], y_sb)

    return nc


def _route(flat, Wr):
    logits = flat @ Wr
    m = logits.max(-1, keepdims=True)
    p = np.exp(logits - m)
    p /= p.sum(-1, keepdims=True)
    topi = np.argsort(-p, axis=-1)[:, :TOP_K]
    topv = np.take_along_axis(p, topi, -1)
    toks, gates = [], []
    for e in range(E):
        sel = topi == e
        rows = np.where(sel.any(-1))[0]
        toks.append(rows)
        gates.append((topv * sel)[rows].sum(-1))
    return toks, gates


_NC_CACHE = {}


def kernel(x, Wr, W1, W2, W3, _trace=False, _result=None):
    x = np.asarray(x)
    Wr = np.asarray(Wr, dtype=np.float32)
    W1 = np.asarray(W1)
    W2 = np.asarray(W2)
    W3 = np.asarray(W3)
    Bx, Tx, Dx = x.shape
    N = Bx * Tx
    flat = np.ascontiguousarray(x.reshape(N, Dx).astype(np.float32))

    toks, gates = _route(flat, Wr)
    counts = np.array([len(t) for t in toks])
    N1, N2 = _solve_split(counts)
    N1 = max(N1, 128)

    # Foreign chunks: per expert, tokens beyond N1 split into <=N2 pieces.
    chunks = []
    for e in range(E):
        rest = toks[e][N1:]
        for s in range(0, len(rest), max(N2, 1)):
            chunks.append((e, rest[s : s + N2]))
    assert len(chunks) <= E, (counts, N1, N2)

    flat16 = flat.astype(NP_DT)
    flat8 = (flat * SX).astype(NP_DT8)
    in_maps = []
    for c in range(E):
        own = toks[c][:N1]
        xte = np.zeros((D, N1), NP_DT)
        xte[:, : len(own)] = flat16[own].T
        w1e = np.zeros((D, HP), NP_DT)
        w1e[:, :H] = W1[c].astype(NP_DT)
        w3e = np.zeros((D, HP), NP_DT)
        w3e[:, :H] = W3[c].astype(NP_DT)
        w2e = np.zeros((HP, D), NP_DT)
        w2e[:H, :] = W2[c].astype(NP_DT)
        im = {"xt": xte, "w1": w1e, "w3": w3e, "w2": w2e}
        if N2:
            xfe = np.zeros((D, N2), NP_DT8)
            w1fe = np.zeros((D, HP), NP_DT8)
            w3fe = np.zeros((D, HP), NP_DT8)
            w2fe = np.zeros((HP, D), NP_DT8)
            if c < len(chunks):
                fe, ftoks = chunks[c]
                xfe[:, : len(ftoks)] = flat8[ftoks].T
                w1fe[:, :H] = (W1[fe] * SW).astype(NP_DT8)
                w3fe[:, :H] = (W3[fe] * SW).astype(NP_DT8)
                w2fe[:H, :] = (W2[fe] * SW2).astype(NP_DT8)
            im.update({"xf": xfe, "w1f": w1fe, "w3f": w3fe, "w2f": w2fe})
        in_maps.append(im)

    key = (N1, N2)
    if key not in _NC_CACHE:
        _NC_CACHE[key] = _build_nc(N1, N2)
    nc = _NC_CACHE[key]

    res = run_bass_kernel_spmd(nc, in_maps, list(range(E)), trace=_trace)
    if _result is not None:
        _result.append(res)

    Y = np.stack([res.results[c]["yt"] for c in range(E)])  # [E, D, N1+N2]
    out = np.zeros((N, D), np.float32)
    for e in range(E):
        own = toks[e][:N1]
        out[own] += gates[e][: len(own), None] * Y[e, :, : len(own)].T
    if N2:
        # gates for foreign tokens: walk each expert's overflow in chunk order
        pos = {e: N1 for e in range(E)}
        for c, (fe, ftoks) in enumerate(chunks):
            k = len(ftoks)
            g = gates[fe][pos[fe] : pos[fe] + k]
            pos[fe] += k
            out[ftoks] += (g[:, None] * Y[c, :, N1 : N1 + k].T) / SW2
    return out.reshape(Bx, Tx, Dx).astype(x.dtype)
